# revision 1
# baseline (speedup 1.0000x reference)
"""CRF negative log-likelihood on 8 Trainium2 NeuronCores.

Strategy
--------
The reference is a CRF forward (log-partition) scan over T=1024 steps plus a
gold-path energy term.  We reformulate the log-space scan in probability
space:  alpha_t = exp(x_t) * (E^T alpha_{t-1})  with E = exp(transition),
so each step is one 64x64 matmul (TensorE) + one elementwise multiply
(VectorE); exp(x) is precomputed on the host (it is not on the recurrence's
critical path).

T is split in half: a forward chain propagates alpha up from t=0 while an
independent backward chain propagates gamma_t = w_t * (E gamma_{t+1}) down
from t=1023; they meet in the middle where Z = alpha_511^T E gamma_512.
Both chains are packed into one [128, b] tile (fwd on partitions 0-63, bwd
on 64-127) driven by a block-diagonal 128x128 weight matrix, halving the
serial depth at no extra instruction cost.

Batch (512) is sharded 8 ways across cores (64 sequences/core, the free
dim).  Within a core the 64 sequences are further split into independent
half-width pipelines whose matmul->multiply round trips interleave on the
engines, hiding each other's latency.  fp32 ranges are kept bounded by
periodic rescaling: a ones-column matmul produces per-sequence sums,
VectorE takes reciprocals, and a rank-1 ones matmul broadcasts them across
partitions; the applied reciprocals are shipped to the host so the
accounting stays exact.

The energy term (pure gathers) and the final tiny combine run on the host
in float64.
"""
import os
import sys
from contextlib import ExitStack

for _p in ("/opt/trn_rl_repo", "/root/.axon_site/_ro/trn_rl_repo"):
    if os.path.isdir(_p) and _p not in sys.path:
        sys.path.append(_p)

import numpy as np
import ml_dtypes

BF16 = ml_dtypes.bfloat16

B, T, F = 512, 1024, 64
NCORE = 8
BL = B // NCORE          # 64 sequences per core (matmul free dim)
TICKS = T // 2 - 1       # 511 serial steps per chain
CHUNK = 64               # ticks per DMA'd weight chunk
NCHUNK = (TICKS + 1) // CHUNK
RENORM = (128, 256, 384)

# NHALF: independent half-pipelines per core (1 or 2).
# SPLIT_MM: use two concurrent K=64 quadrant matmuls instead of one K=128.
NHALF = int(os.environ.get("CRF_NHALF", "2"))
SPLIT_MM = os.environ.get("CRF_SPLIT_MM", "0") == "1"
HB = BL // NHALF

_PROG = None
LAST_EXEC_NS = None
LAST_RESULTS = None


def _patch_ldw_opt():
    """The recurrence reuses one stationary weight matrix for every matmul;
    let walrus drop the redundant per-matmul LDWEIGHTS (off by default)."""
    import concourse.bass_utils as bu

    if getattr(bu, "_crf_ldw_patched", False):
        return
    # NOTE: --enable-ldw-opt=true crashes this walrus build
    # (visitInstLdweights, CoreV3GenImpl.cpp:694) — leave the flag alone.
    bu._crf_ldw_patched = True


def _build_program():
    import concourse.bacc as bacc
    import concourse.tile as tile
    from concourse import mybir

    _patch_ldw_opt()

    dt = mybir.dt
    nc = bacc.Bacc("TRN2", target_bir_lowering=False, debug=False)
    w_d = nc.dram_tensor("w", [NCHUNK, 128, CHUNK * BL], dt.bfloat16,
                         kind="ExternalInput")
    wmat_d = nc.dram_tensor("wmat", [128, 128], dt.bfloat16,
                            kind="ExternalInput")
    state_d = nc.dram_tensor("state", [128, BL], dt.bfloat16,
                             kind="ExternalOutput")
    rstage_d = nc.dram_tensor("rstage", [128, len(RENORM) * BL], dt.bfloat16,
                              kind="ExternalOutput")

    with tile.TileContext(nc) as tc, nc.allow_low_precision(
            reason="bf16 state is within tolerance (validated vs reference)"):
        with ExitStack() as ctx:
            wpool = ctx.enter_context(tc.tile_pool(name="wst", bufs=3))
            spool = ctx.enter_context(tc.tile_pool(name="state", bufs=3))
            cpool = ctx.enter_context(tc.tile_pool(name="const", bufs=1))
            qpool = ctx.enter_context(tc.tile_pool(name="q", bufs=3, space="PSUM"))
            rpool = ctx.enter_context(tc.tile_pool(name="ren", bufs=1, space="PSUM"))

            wmat_sb = cpool.tile([128, 128], dt.bfloat16)
            nc.sync.dma_start(wmat_sb[:, :], wmat_d[:, :])
            ones_sb = cpool.tile([128, BL], dt.bfloat16)
            nc.vector.memset(ones_sb[:, :], 1.0)
            rstage_sb = cpool.tile([128, len(RENORM) * BL], dt.bfloat16)

            def chunk_tile(c):
                t = wpool.tile([128, CHUNK * BL], dt.bfloat16, tag="wchunk")
                nc.sync.dma_start(t[:, :], w_d[c, :, :])
                return t

            def do_mm(q, state):
                if SPLIT_MM:
                    # two K=64 matmuls in disjoint PE array quadrants -> they
                    # run concurrently and each drains in ~half the time
                    nc.tensor.matmul(q[0:64, :], wmat_sb[0:64, 0:64],
                                     state[0:64, :], start=True, stop=True,
                                     tile_position=(0, 0))
                    nc.tensor.matmul(q[64:128, :], wmat_sb[64:128, 64:128],
                                     state[64:128, :], start=True, stop=True,
                                     tile_position=(64, 64))
                else:
                    nc.tensor.matmul(q[:, :], wmat_sb[:, :], state[:, :],
                                     start=True, stop=True)

            wt = chunk_tile(0)
            states = []
            for h in range(NHALF):
                st = spool.tile([128, HB], dt.bfloat16, tag=f"state{h}")
                nc.vector.tensor_copy(st[:, :], wt[:, h * HB:(h + 1) * HB])
                states.append(st)

            ren_i = 0
            for tau in range(1, TICKS + 1):
                c, sl = divmod(tau, CHUNK)
                if sl == 0:
                    wt = chunk_tile(c)
                for h in range(NHALF):
                    q = qpool.tile([128, HB], dt.float32, tag=f"q{h}")
                    do_mm(q, states[h])
                    st_new = spool.tile([128, HB], dt.bfloat16, tag=f"state{h}")
                    nc.vector.tensor_mul(
                        st_new[:, :], q[:, :],
                        wt[:, sl * BL + h * HB: sl * BL + (h + 1) * HB])
                    states[h] = st_new
                if tau in RENORM:
                    for h in range(NHALF):
                        state = states[h]
                        sr = rpool.tile([128, HB], dt.float32, tag="sr")
                        nc.tensor.matmul(sr[64:65, :], ones_sb[0:64, 0:1],
                                         state[0:64, :], start=True, stop=True,
                                         tile_position=(0, 64))
                        nc.tensor.matmul(sr[0:1, :], ones_sb[64:128, 0:1],
                                         state[64:128, :], start=True, stop=True,
                                         tile_position=(64, 0))
                        lo = ren_i * BL + h * HB
                        rsl = rstage_sb[:, lo:lo + HB]
                        nc.vector.reciprocal(rsl[64:65, :], sr[64:65, :])
                        nc.vector.reciprocal(rsl[0:1, :], sr[0:1, :])
                        bc = rpool.tile([128, HB], dt.float32, tag="bc")
                        nc.tensor.matmul(bc[0:64, :], ones_sb[64:65, 0:64],
                                         rsl[64:65, :], start=True, stop=True,
                                         tile_position=(64, 0))
                        nc.tensor.matmul(bc[64:128, :], ones_sb[0:1, 0:64],
                                         rsl[0:1, :], start=True, stop=True,
                                         tile_position=(0, 64))
                        st_rn = spool.tile([128, HB], dt.bfloat16,
                                           tag=f"state{h}")
                        nc.vector.tensor_mul(st_rn[:, :], state[:, :], bc[:, :])
                        states[h] = st_rn
                    ren_i += 1

            for h in range(NHALF):
                nc.sync.dma_start(state_d[:, h * HB:(h + 1) * HB],
                                  states[h][:, :])
            nc.sync.dma_start(rstage_d[:, :], rstage_sb[:, :])

    nc.compile()
    return nc


def _build_program_bacc():
    """Hand-scheduled variant: manual semaphores, fused waits/incs, explicit
    PSUM bank rotation.  Two independent half-width (FD=32) pipelines whose
    matmul->multiply round trips interleave on TensorE/VectorE."""
    import concourse.bacc as bacc
    from concourse import mybir

    dt = mybir.dt
    assert NHALF == 2
    nc = bacc.Bacc("TRN2", target_bir_lowering=False, debug=False)
    w_d = nc.dram_tensor("w", [NCHUNK, 128, CHUNK * BL], dt.bfloat16,
                         kind="ExternalInput")
    wmat_d = nc.dram_tensor("wmat", [128, 128], dt.bfloat16,
                            kind="ExternalInput")
    state_d = nc.dram_tensor("state", [128, BL], dt.bfloat16,
                             kind="ExternalOutput")
    rstage_d = nc.dram_tensor("rstage", [128, len(RENORM) * BL], dt.bfloat16,
                              kind="ExternalOutput")

    NSLOT = 4    # SBUF state slots per half
    NQ = 3       # PSUM q banks per half

    wmat_sb = nc.alloc_sbuf_tensor("wmat_sb", [128, 128], dt.bfloat16)
    ones_sb = nc.alloc_sbuf_tensor("ones_sb", [128, BL], dt.bfloat16)
    rstage_sb = nc.alloc_sbuf_tensor("rstage_sb", [128, len(RENORM) * BL],
                                     dt.bfloat16)
    wbuf = [nc.alloc_sbuf_tensor(f"wbuf{i}", [128, CHUNK * BL], dt.bfloat16)
            for i in range(3)]
    stslot = [[nc.alloc_sbuf_tensor(f"st{h}_{s}", [128, HB], dt.bfloat16)
               for s in range(NSLOT)] for h in range(2)]
    qslot = [[nc.place_psum_tensor(f"q{h}_{s}", [128, HB], dt.float32,
                                   bank=h * NQ + s) for s in range(NQ)]
             for h in range(2)]
    sr_ps = nc.place_psum_tensor("sr_ps", [128, HB], dt.float32, bank=6)
    bc_ps = nc.place_psum_tensor("bc_ps", [128, HB], dt.float32, bank=7)

    def mm_pair(out, lhsT, rhs, wait=None, tile_position=None):
        # explicit Ldweights (no wait -> silicon pulls it ahead into the
        # background weight buffer) + a non-self-loading Matmult carrying
        # the data dependency wait
        nc.tensor.ldweights(lhsT, tile_position=tile_position)
        mm = nc.tensor.matmul(out, lhsT, rhs, start=True, stop=True,
                              tile_position=tile_position)
        mm.ins.ldweights = False
        if wait is not None:
            mm._wait_ge(*wait)
        return mm.then_inc(pe_sem)

    pe_sem = nc.alloc_semaphore("pe_sem")
    dve_sem = nc.alloc_semaphore("dve_sem")
    dma_sem = nc.alloc_semaphore("dma_sem")

    with nc.allow_low_precision(reason="bf16 state validated vs reference"):
        pe_n = 0
        dve_n = 0
        # ---- DMA engine program (sync): wmat, then chunk stream ----
        nc.sync.dma_start(wmat_sb[:, :], wmat_d[:, :]).then_inc(dma_sem, 16)
        chunk_end_tt = {}   # chunk -> dve_sem count that releases its buffer
        for c in range(3):
            nc.sync.dma_start(wbuf[c][:, :], w_d[c, :, :]).then_inc(dma_sem, 16)
        # remaining chunks are emitted lazily below once their buffer frees

        # ---- init: ones + state copies ----
        nc.vector.memset(ones_sb[:, :], 1.0)
        nc.vector.wait_ge(dma_sem, 32)          # wmat + chunk0 landed
        last_tt = [None, None]
        cur = [0, 0]                            # current state slot per half
        for h in range(2):
            nc.vector.tensor_copy(
                stslot[h][0][:, :], wbuf[0][:, h * HB:(h + 1) * HB]
            ).then_inc(dve_sem)
            dve_n += 1
            last_tt[h] = dve_n
        mm_of = [None, None]                    # pe_sem count of half's live mm
        ren_i = 0
        pe_first = True

        for tau in range(1, TICKS + 1):
            c, sl = divmod(tau, CHUNK)
            if sl == 0 and c + 2 < NCHUNK:
                # prefetch chunk c+2 into the buffer freed by chunk c-1
                if c - 1 in chunk_end_tt:
                    nc.sync.wait_ge(dve_sem, chunk_end_tt[c - 1])
                nc.sync.dma_start(wbuf[(c + 2) % 3][:, :],
                                  w_d[c + 2, :, :]).then_inc(dma_sem, 16)
            # ---- PE: one matmul per half ----
            for h in range(2):
                if pe_first:
                    nc.tensor.wait_ge(dma_sem, 16)   # wmat resident
                    pe_first = False
                q = qslot[h][tau % NQ]
                st_cur = stslot[h][cur[h]]
                if SPLIT_MM:
                    # two K=64 matmuls in disjoint quadrants run concurrently
                    # and drain through half the array depth
                    mm_pair(q[0:64, :], wmat_sb[0:64, 0:64],
                            st_cur[0:64, :], wait=(dve_sem, last_tt[h]),
                            tile_position=(0, 0))
                    pe_n += 1
                    mm_pair(q[64:128, :], wmat_sb[64:128, 64:128],
                            st_cur[64:128, :], wait=(dve_sem, last_tt[h]),
                            tile_position=(64, 64))
                    pe_n += 1
                else:
                    mm_pair(q[:, :], wmat_sb[:, :], st_cur[:, :],
                            wait=(dve_sem, last_tt[h]))
                    pe_n += 1
                mm_of[h] = pe_n
            # ---- DVE: multiply per half ----
            for h in range(2):
                if h == 0 and sl == 0 and c > 0:
                    nc.vector.wait_ge(dma_sem, 16 * (c + 2))  # chunk c landed
                nxt = (cur[h] + 1) % NSLOT
                nc.vector.tensor_mul(
                    stslot[h][nxt][:, :], qslot[h][tau % NQ][:, :],
                    wbuf[c % 3][:, sl * BL + h * HB: sl * BL + (h + 1) * HB]
                )._wait_ge(pe_sem, mm_of[h]).then_inc(dve_sem)
                dve_n += 1
                cur[h] = nxt
                last_tt[h] = dve_n
            if sl == CHUNK - 1 or tau == TICKS:
                chunk_end_tt[c] = dve_n
            # ---- renorm ----
            if tau in RENORM:
                for h in range(2):
                    st = stslot[h][cur[h]]
                    mm_pair(sr_ps[64:65, :], ones_sb[0:64, 0:1],
                            st[0:64, :], wait=(dve_sem, last_tt[h]),
                            tile_position=(0, 64))
                    pe_n += 1
                    mm_pair(sr_ps[0:1, :], ones_sb[64:128, 0:1],
                            st[64:128, :], tile_position=(64, 0))
                    pe_n += 1
                    lo = ren_i * BL + h * HB
                    rsl = rstage_sb[:, lo:lo + HB]
                    nc.vector.reciprocal(rsl[64:65, :],
                                         sr_ps[64:65, :])._wait_ge(
                        pe_sem, pe_n).then_inc(dve_sem)
                    dve_n += 1
                    nc.vector.reciprocal(rsl[0:1, :],
                                         sr_ps[0:1, :]).then_inc(dve_sem)
                    dve_n += 1
                    mm_pair(bc_ps[0:64, :], ones_sb[64:65, 0:64],
                            rsl[64:65, :], wait=(dve_sem, dve_n),
                            tile_position=(64, 0))
                    pe_n += 1
                    mm_pair(bc_ps[64:128, :], ones_sb[0:1, 0:64],
                            rsl[0:1, :], tile_position=(0, 64))
                    pe_n += 1
                    nxt = (cur[h] + 1) % NSLOT
                    nc.vector.tensor_mul(stslot[h][nxt][:, :], st[:, :],
                                         bc_ps[:, :])._wait_ge(
                        pe_sem, pe_n).then_inc(dve_sem)
                    dve_n += 1
                    cur[h] = nxt
                    last_tt[h] = dve_n
                ren_i += 1

        # ---- tail: ship state + rstage ----
        nc.sync.wait_ge(dve_sem, dve_n)
        for h in range(2):
            nc.sync.dma_start(state_d[:, h * HB:(h + 1) * HB],
                              stslot[h][cur[h]][:, :]).then_inc(dma_sem, 16)
        nc.sync.dma_start(rstage_d[:, :], rstage_sb[:, :]).then_inc(dma_sem, 16)

    nc.compile()
    return nc


def _get_program():
    global _PROG
    if _PROG is None:
        if os.environ.get("CRF_IMPL", "tile") == "bacc":
            _PROG = _build_program_bacc()
        else:
            _PROG = _build_program()
    return _PROG


def _install_ntff_hook():
    """Recreate antenv.axon_hooks (absent from this image) so trace=True can
    capture NTFF profiles through the axon PJRT .so."""
    import types, ctypes, contextlib

    so_path = "/opt/axon/libaxon_pjrt.so"
    if "antenv.axon_hooks" in sys.modules or not os.path.exists(so_path):
        return
    lib = ctypes.CDLL(so_path)
    if not hasattr(lib, "axon_start_nrt_profile"):
        return
    lib.axon_start_nrt_profile.argtypes = [ctypes.POINTER(ctypes.c_int64),
                                           ctypes.c_size_t]
    lib.axon_start_nrt_profile.restype = ctypes.c_int64
    lib.axon_stop_nrt_profile.argtypes = [ctypes.c_char_p]
    lib.axon_stop_nrt_profile.restype = ctypes.c_int64

    @contextlib.contextmanager
    def _hook(output_dir, device_ids):
        import jax

        jax.devices()
        if device_ids:
            ids = (ctypes.c_int64 * len(device_ids))(*device_ids)
            rc = lib.axon_start_nrt_profile(ids, len(device_ids))
        else:
            rc = lib.axon_start_nrt_profile(None, 0)
        if rc != 0:
            raise RuntimeError(f"axon_start_nrt_profile rc={rc}")
        try:
            yield
        finally:
            n = lib.axon_stop_nrt_profile(str(output_dir).encode())
            print(f"profile: {n} file(s) written to {output_dir}")

    mod = types.ModuleType("antenv.axon_hooks")
    mod.get_axon_ntff_profile_hook = lambda: _hook
    mod.set_axon_ntff_profile_hook = lambda h: None
    sys.modules["antenv.axon_hooks"] = mod


def _host_energy(x, mask, y_true, transition):
    x64 = x.astype(np.float64)
    m64 = mask.astype(np.float64)
    y = y_true.astype(np.int64)
    ie = np.take_along_axis(x64, y[..., None], axis=2)[..., 0] * m64
    ce = transition.astype(np.float64)[y[:, :-1], y[:, 1:]] * (
        m64[:, :-1] * m64[:, 1:])
    return ie.sum(1) + ce.sum(1)


def _host_fallback(x, mask, y_true, transition):
    """Exact float64 port of the reference, used only if mask isn't all-ones
    (the device scan bakes in unit masks)."""
    x64 = x.astype(np.float64)
    m64 = mask.astype(np.float64)
    Tm = transition.astype(np.float64)
    state = x64[:, 0, :]
    for t in range(1, T):
        e_t = x64[:, t, :] * m64[:, t][:, None]
        chain = e_t[:, None, :] + Tm[None, :, :]
        chain = chain * (m64[:, t - 1] * m64[:, t])[:, None, None]
        score = state[:, :, None] + chain
        mx = score.max(axis=1)
        state = np.log(np.exp(score - mx[:, None, :]).sum(axis=1)) + mx
    mx = state.max(axis=1)
    logZ = np.log(np.exp(state - mx[:, None]).sum(axis=1)) + mx
    energy = _host_energy(x, mask, y_true, transition)
    nll = (logZ - energy) / m64.sum(1)
    return np.asarray(nll.sum() / B, dtype=np.float32)


def kernel(x, mask, y_true, transition):
    from concourse.bass_utils import run_bass_kernel_spmd

    x = np.ascontiguousarray(np.asarray(x, dtype=np.float32))
    mask = np.asarray(mask, dtype=np.float32)
    transition = np.asarray(transition, dtype=np.float32)
    y_true = np.asarray(y_true)
    assert x.shape == (B, T, F), x.shape

    if not np.all(mask == 1.0):
        return _host_fallback(x, mask, y_true, transition)

    E64 = np.exp(transition.astype(np.float64))
    c_E = E64.sum(0).mean() * np.exp(0.5)
    Epp = (E64 / c_E).astype(BF16)
    wmat = np.zeros((128, 128), dtype=BF16)
    wmat[0:64, 0:64] = Epp                # lhsT[i, j] = E''[i, j]  (fwd)
    wmat[64:128, 64:128] = Epp.T          # lhsT[64+j, 64+i] = E''[i, j] (bwd)

    ex = np.exp(x)                        # [B, T, F] fp32
    in_maps = []
    for c in range(NCORE):
        xb = ex[c * BL:(c + 1) * BL]                       # [BL, T, F]
        fwd = xb.transpose(1, 2, 0)[:TICKS + 1]            # [512, F, BL]
        bwd = xb[:, ::-1].transpose(1, 2, 0)[:TICKS + 1]   # [512, F, BL]
        W = np.concatenate([fwd, bwd], axis=1)             # [512, 128, BL]
        W = W.reshape(NCHUNK, CHUNK, 128, BL).transpose(0, 2, 1, 3)
        W = np.ascontiguousarray(W.reshape(NCHUNK, 128, CHUNK * BL)).astype(BF16)
        in_maps.append({"w": W, "wmat": wmat})

    nc = _get_program()
    trace = os.environ.get("CRF_TRACE") == "1"
    if trace:
        _install_ntff_hook()
    res = run_bass_kernel_spmd(nc, in_maps, list(range(NCORE)), trace=trace)
    global LAST_EXEC_NS, LAST_RESULTS
    LAST_EXEC_NS = res.exec_time_ns
    LAST_RESULTS = res

    logZ = np.empty(B, dtype=np.float64)
    corr = 2 * TICKS * np.log(c_E)
    for c in range(NCORE):
        st = res.results[c]["state"].astype(np.float64)    # [128, BL]
        rs = res.results[c]["rstage"].astype(np.float64)   # [128, NREN*BL]
        af, gf = st[0:64], st[64:128]
        dot = np.einsum("ib,ij,jb->b", af, E64, gf)
        r_log = np.zeros(BL, dtype=np.float64)
        for k in range(len(RENORM)):
            r_log -= np.log(rs[64, k * BL:(k + 1) * BL])   # fwd reciprocals
            r_log -= np.log(rs[0, k * BL:(k + 1) * BL])    # bwd reciprocals
        logZ[c * BL:(c + 1) * BL] = np.log(dot) + corr + r_log

    energy = _host_energy(x, mask, y_true, transition)
    denom = mask.astype(np.float64).sum(1)
    nll = (logZ - energy) / denom
    return np.asarray(nll.sum() / B, dtype=np.float32)



# revision 2
# speedup vs baseline: 4.1852x; 4.1852x over previous
"""CRF negative log-likelihood on 8 Trainium2 NeuronCores.

Strategy (v2: overlapped telescoping segments)
----------------------------------------------
The reference is a CRF forward (log-partition) scan over T=1024 steps plus
a gold-path energy term.  In probability space the scan is
alpha_t = w_t * (E^T alpha_{t-1}) with w_t = exp(x_t), E = exp(transition).

E's entries are all ~1 (xavier-scaled transition), so A_t = diag(w_t) E^T
contracts the projective (Hilbert) metric by ~0.02 per step: any positive
probe vector converges to the true alpha direction in a few steps.  That
lets us break the serial scan into S=64 independent chains per core, each
owning L=16 steps plus V=3 burn-in steps from a ones-probe.  Per-segment
log-growth ratios (1^T alpha at segment end / start) then telescope into
logZ with splice error ~kappa^V ~ 1e-5, far below the bf16 noise floor.

Serial depth drops 511 -> 20 ticks, so the kernel becomes throughput-bound
and the work is spread across engines: chains are packed two-per-partition-
half into a [128, 2048] working set split into 4 column streams.  Stream 0
runs matmul -> DVE multiply (PSUM source); streams 1-3 run DVE bf16
multiply -> matmul -> ScalarE PSUM->SBUF copy, which moves the PSUM
evacuation onto the otherwise idle ScalarE and lets the DVE multiplies hit
the 2x bf16 SBUF mode.  For those streams the multiply output *is* alpha,
so snapshots ship the multiply tile.  Three snapshot DMAs (after burn-in,
and at the two final ticks) give the host everything needed to assemble
logZ in float64.

Batch (512) is sharded 8 ways across cores (64 sequences/core).  The
energy term (pure gathers) and the final splice run on the host in f64.
"""
import os
import sys
from contextlib import ExitStack

for _p in ("/opt/trn_rl_repo", "/root/.axon_site/_ro/trn_rl_repo"):
    if os.path.isdir(_p) and _p not in sys.path:
        sys.path.append(_p)

import numpy as np
import ml_dtypes

BF16 = ml_dtypes.bfloat16

B, T, F = 512, 1024, 64
NCORE = 8
BL = B // NCORE            # 64 sequences per core

S_SEG = int(os.environ.get("CRF_S", "64"))   # chains (segments) per core
V_BURN = int(os.environ.get("CRF_V", "3"))   # burn-in steps per chain
L_SEG = T // S_SEG                            # owned steps per chain
NT = V_BURN + L_SEG + 1                       # ticks: 0 = init, 1..NT-1 compute
NTICK = NT - 1                                # weight slices
NBLK = S_SEG // 2                             # 64-col blocks (2 chains/block)
W = NBLK * BL                                 # free width of the working set
NSTR = int(os.environ.get("CRF_NSTR", "4"))   # column streams
P1_STREAMS = frozenset(
    int(t) for t in os.environ.get("CRF_P1", "0").split(",") if t != "")
# stream column extents (in blocks); default: equal split
_blk_per = [NBLK // NSTR + (1 if i < NBLK % NSTR else 0) for i in range(NSTR)]
STR_LO = [sum(_blk_per[:i]) * BL for i in range(NSTR)]
STR_HI = [sum(_blk_per[:i + 1]) * BL for i in range(NSTR)]

_PROG = None
LAST_EXEC_NS = None
LAST_RESULTS = None


def _build_program():
    import concourse.bacc as bacc
    import concourse.tile as tile
    from concourse import mybir

    dt = mybir.dt
    nc = bacc.Bacc("TRN2", target_bir_lowering=False, debug=False)
    w_d = nc.dram_tensor("w", [NTICK, 128, W], dt.bfloat16,
                         kind="ExternalInput")
    wmat_d = nc.dram_tensor("wmat", [128, 128], dt.bfloat16,
                            kind="ExternalInput")
    snapb_d = nc.dram_tensor("snapb", [128, W], dt.bfloat16,
                             kind="ExternalOutput")
    snapm2_d = nc.dram_tensor("snapm2", [128, W], dt.bfloat16,
                              kind="ExternalOutput")
    snapm1_d = nc.dram_tensor("snapm1", [128, W], dt.bfloat16,
                              kind="ExternalOutput")
    snap_of = {V_BURN: snapb_d, NT - 2: snapm2_d, NT - 1: snapm1_d}

    with tile.TileContext(nc) as tc, nc.allow_low_precision(
            reason="bf16 state is within tolerance (validated vs reference)"):
        with ExitStack() as ctx:
            wpool = ctx.enter_context(tc.tile_pool(name="wst", bufs=3))
            spool = ctx.enter_context(tc.tile_pool(name="state", bufs=3))
            mpool = ctx.enter_context(tc.tile_pool(name="mtile", bufs=3))
            cpool = ctx.enter_context(tc.tile_pool(name="const", bufs=1))
            qpools = [ctx.enter_context(
                tc.tile_pool(name=f"q{i}", bufs=2, space="PSUM"))
                for i in range(NSTR)]

            wmat_sb = cpool.tile([128, 128], dt.bfloat16)
            nc.sync.dma_start(wmat_sb[:, :], wmat_d[:, :])

            states = []
            for st in range(NSTR):
                t0 = spool.tile([128, STR_HI[st] - STR_LO[st]], dt.bfloat16,
                                tag=f"s{st}")
                nc.vector.memset(t0[:, :], 1.0)
                states.append(t0)

            def fetch(j):
                t = wpool.tile([128, W], dt.bfloat16, tag="wchunk")
                nc.sync.dma_start(t[:, :], w_d[j - 1, :, :])
                return t

            wts = {}
            for j in range(1, min(3, NT)):
                wts[j] = fetch(j)

            for j in range(1, NT):
                if j + 2 <= NT - 1:
                    wts[j + 2] = fetch(j + 2)
                wt = wts.pop(j)
                snap_d = snap_of.get(j)
                snap_tiles = [None] * NSTR
                # phase-1 matmuls first: they only need last tick's state
                for st in range(NSTR):
                    if st in P1_STREAMS:
                        ws = STR_HI[st] - STR_LO[st]
                        q = qpools[st].tile([128, ws], dt.float32, tag="q")
                        nc.tensor.matmul(q[:, :], wmat_sb[:, :],
                                         states[st][:, :],
                                         start=True, stop=True)
                        states[st] = (q, None)
                # phase-2 multiplies (bf16 SBUF, 2x mode)
                for st in range(NSTR):
                    if st not in P1_STREAMS:
                        ws = STR_HI[st] - STR_LO[st]
                        m = mpool.tile([128, ws], dt.bfloat16, tag=f"m{st}")
                        nc.vector.tensor_mul(
                            m[:, :], states[st][:, :],
                            wt[:, STR_LO[st]:STR_HI[st]])
                        states[st] = (states[st], m)
                        snap_tiles[st] = m
                # phase-2 matmuls + ScalarE copies
                for st in range(NSTR):
                    if st not in P1_STREAMS:
                        ws = STR_HI[st] - STR_LO[st]
                        _, m = states[st]
                        q = qpools[st].tile([128, ws], dt.float32, tag="q")
                        nc.tensor.matmul(q[:, :], wmat_sb[:, :], m[:, :],
                                         start=True, stop=True)
                        s_new = spool.tile([128, ws], dt.bfloat16,
                                           tag=f"s{st}")
                        nc.scalar.copy(s_new[:, :], q[:, :])
                        states[st] = s_new
                # phase-1 multiplies (PSUM source)
                for st in range(NSTR):
                    if st in P1_STREAMS:
                        ws = STR_HI[st] - STR_LO[st]
                        q, _ = states[st]
                        s_new = spool.tile([128, ws], dt.bfloat16,
                                           tag=f"s{st}")
                        nc.vector.tensor_mul(
                            s_new[:, :], q[:, :],
                            wt[:, STR_LO[st]:STR_HI[st]])
                        states[st] = s_new
                        snap_tiles[st] = s_new
                if snap_d is not None:
                    for st in range(NSTR):
                        nc.sync.dma_start(
                            snap_d[:, STR_LO[st]:STR_HI[st]],
                            snap_tiles[st][:, :])

    nc.compile()
    return nc


def _get_program():
    global _PROG
    if _PROG is None:
        _PROG = _build_program()
    return _PROG


def _install_ntff_hook():
    """Recreate antenv.axon_hooks (absent from this image) so trace=True can
    capture NTFF profiles through the axon PJRT .so."""
    import types, ctypes, contextlib

    so_path = "/opt/axon/libaxon_pjrt.so"
    if "antenv.axon_hooks" in sys.modules or not os.path.exists(so_path):
        return
    lib = ctypes.CDLL(so_path)
    if not hasattr(lib, "axon_start_nrt_profile"):
        return
    lib.axon_start_nrt_profile.argtypes = [ctypes.POINTER(ctypes.c_int64),
                                           ctypes.c_size_t]
    lib.axon_start_nrt_profile.restype = ctypes.c_int64
    lib.axon_stop_nrt_profile.argtypes = [ctypes.c_char_p]
    lib.axon_stop_nrt_profile.restype = ctypes.c_int64

    @contextlib.contextmanager
    def _hook(output_dir, device_ids):
        import jax

        jax.devices()
        if device_ids:
            ids = (ctypes.c_int64 * len(device_ids))(*device_ids)
            rc = lib.axon_start_nrt_profile(ids, len(device_ids))
        else:
            rc = lib.axon_start_nrt_profile(None, 0)
        if rc != 0:
            raise RuntimeError(f"axon_start_nrt_profile rc={rc}")
        try:
            yield
        finally:
            n = lib.axon_stop_nrt_profile(str(output_dir).encode())
            print(f"profile: {n} file(s) written to {output_dir}")

    mod = types.ModuleType("antenv.axon_hooks")
    mod.get_axon_ntff_profile_hook = lambda: _hook
    mod.set_axon_ntff_profile_hook = lambda h: None
    sys.modules["antenv.axon_hooks"] = mod


def _host_energy(x, mask, y_true, transition):
    x64 = x.astype(np.float64)
    m64 = mask.astype(np.float64)
    y = y_true.astype(np.int64)
    ie = np.take_along_axis(x64, y[..., None], axis=2)[..., 0] * m64
    ce = transition.astype(np.float64)[y[:, :-1], y[:, 1:]] * (
        m64[:, :-1] * m64[:, 1:])
    return ie.sum(1) + ce.sum(1)


def _host_fallback(x, mask, y_true, transition):
    """Exact float64 port of the reference, used only if mask isn't all-ones
    (the device scan bakes in unit masks)."""
    x64 = x.astype(np.float64)
    m64 = mask.astype(np.float64)
    Tm = transition.astype(np.float64)
    state = x64[:, 0, :]
    for t in range(1, T):
        e_t = x64[:, t, :] * m64[:, t][:, None]
        chain = e_t[:, None, :] + Tm[None, :, :]
        chain = chain * (m64[:, t - 1] * m64[:, t])[:, None, None]
        score = state[:, :, None] + chain
        mx = score.max(axis=1)
        state = np.log(np.exp(score - mx[:, None, :]).sum(axis=1)) + mx
    mx = state.max(axis=1)
    logZ = np.log(np.exp(state - mx[:, None]).sum(axis=1)) + mx
    energy = _host_energy(x, mask, y_true, transition)
    nll = (logZ - energy) / m64.sum(1)
    return np.asarray(nll.sum() / B, dtype=np.float32)


def _chain_loc(s):
    """chain s -> (partition half, column block)."""
    return s % 2, s // 2


def _build_weight_stream(ex_core, cvec):
    """ex_core: [BL, T, F] f32 exp(x) for one core; cvec: f64 E''^T @ 1.
    Returns [NTICK, 128, W] bf16 tick-major weight stream."""
    Wst = np.empty((NTICK, 128, W), dtype=BF16)
    inv_c = (1.0 / cvec).astype(np.float32)          # [F]
    ones_col = np.ones((BL, F), dtype=np.float32)
    for s in range(S_SEG):
        h, blk = _chain_loc(s)
        rows = slice(h * 64, h * 64 + 64)
        cols = slice(blk * BL, (blk + 1) * BL)
        base = s * L_SEG - V_BURN
        for j in range(1, NT):
            t = base + j
            if s == 0 and j < V_BURN:
                sl = np.broadcast_to(inv_c[:, None], (F, BL))
            elif s == 0 and j == V_BURN:
                sl = (ex_core[:, 0, :] * inv_c[None, :]).T
            elif t >= T:
                sl = ones_col.T
            else:
                sl = ex_core[:, t, :].T               # [F, BL]
            Wst[j - 1, rows, cols] = sl.astype(BF16)
    return Wst


def kernel(x, mask, y_true, transition):
    from concourse.bass_utils import run_bass_kernel_spmd

    x = np.ascontiguousarray(np.asarray(x, dtype=np.float32))
    mask = np.asarray(mask, dtype=np.float32)
    transition = np.asarray(transition, dtype=np.float32)
    y_true = np.asarray(y_true)
    assert x.shape == (B, T, F), x.shape

    if not np.all(mask == 1.0):
        return _host_fallback(x, mask, y_true, transition)

    E64 = np.exp(transition.astype(np.float64))
    c_E = E64.sum(0).mean() * np.exp(0.5)
    Epp = (E64 / c_E).astype(BF16)
    Epp64 = Epp.astype(np.float64)
    cvec = Epp64.sum(0)                    # E''^T @ ones (device colsums)
    wmat = np.zeros((128, 128), dtype=BF16)
    wmat[0:64, 0:64] = Epp                 # lhsT = E'' -> out = E''^T @ state
    wmat[64:128, 64:128] = Epp             # both halves run forward chains

    ex = np.exp(x)                         # [B, T, F] f32
    in_maps = []
    for c in range(NCORE):
        Wst = _build_weight_stream(ex[c * BL:(c + 1) * BL], cvec)
        in_maps.append({"w": Wst, "wmat": wmat})

    nc = _get_program()
    trace = os.environ.get("CRF_TRACE") == "1"
    if trace:
        _install_ntff_hook()
    res = run_bass_kernel_spmd(nc, in_maps, list(range(NCORE)), trace=trace)
    global LAST_EXEC_NS, LAST_RESULTS
    LAST_EXEC_NS = res.exec_time_ns
    LAST_RESULTS = res

    # ---- host splice (f64): telescoped per-segment log growth ----
    log_cE = np.log(c_E)
    nsteps = np.full(S_SEG, L_SEG, dtype=np.float64)
    nsteps[S_SEG - 1] = L_SEG - 1
    logZ = np.empty(B, dtype=np.float64)
    for c in range(NCORE):
        snapb = res.results[c]["snapb"].astype(np.float64)     # [128, W]
        snapm2 = res.results[c]["snapm2"].astype(np.float64)
        snapm1 = res.results[c]["snapm1"].astype(np.float64)
        lz = np.log(ex[c * BL:(c + 1) * BL, 0, :].astype(np.float64).sum(1))
        for s in range(S_SEG):
            h, blk = _chain_loc(s)
            rows = slice(h * 64, h * 64 + 64)
            cols = slice(blk * BL, (blk + 1) * BL)
            bsum = snapb[rows, cols].sum(0)                    # [BL]
            msrc = snapm2 if s == S_SEG - 1 else snapm1
            msum = msrc[rows, cols].sum(0)
            lz += np.log(msum) - np.log(bsum) + nsteps[s] * log_cE
        logZ[c * BL:(c + 1) * BL] = lz

    energy = _host_energy(x, mask, y_true, transition)
    denom = mask.astype(np.float64).sum(1)
    nll = (logZ - energy) / denom
    return np.asarray(nll.sum() / B, dtype=np.float32)
